# revision 20
# baseline (speedup 1.0000x reference)
"""Trainium2 Bass kernel for nn_ModalityMoERouter (expert-choice MoE routing).

Contract: kernel(**inputs) takes the FULL inputs from reference.setup_inputs()
and returns (dispatch, combine), each (16, 8192, 16) float32.

Sharding: data-parallel over batch B=16 across 8 NeuronCores (2 batches/core);
gate weights and expert centers replicated. The global mean(dists) scalar is
computed with one AllReduce (overlapped with the gate MLP).

Key design points vs the fp32 baseline (667 us):
 - All large matmuls run in float32r (1 cycle/row on TRN2 vs 4 for fp32;
   ~11-bit mantissa, rel err ~1.5e-4 -- measured on HW; final output rel err
   ~1.4e-2, within the 2e-2 gate).
 - Tokens are transposed on the HOST (numpy) to [D, N] layout, so the kernel
   needs zero PE transposes and no PSUM-evacuation copies for activations.
 - dists(token, e) come from ONE small fp32r matmul per 512-token block using
   rows [x, y, z, |x|^2, 1] against columns [-2cx, -2cy, -2cz, 1, |c|^2];
   sqrt on ACT with accum_out gives the global-mean numerator for free.
 - Expert-choice top-k threshold via quaternary bisection (3 compares/round,
   11 rounds) using tensor_scalar is_gt + accum_out (2x_2P DVE mode).
 - Outputs are written in the on-chip (g,e)-partition layout and untransposed
   on the host.

On-chip layout: [128, 1024] per batch with partition p = g*16+e and free
f = blk*512 + t, where token n = (blk*8+g)*512 + t.
The hard-cap step of the reference is a bitwise no-op (dispatch <= 0.4375 <
cap >= 0.5), so it is skipped (t unused).
"""

import numpy as np

B = 16
N = 8192
D = 512
H = 256
E = 16
N_CORES = 8
BPC = B // N_CORES
KSEL = N * 2 // E           # 1024
ALPHA = min(min(0.05, 0.15 / 4) * E, 1.0)
DSCALE = 1.0 - ALPHA        # 0.4
DFLOOR = ALPHA / E          # 0.0375
N_ROUNDS = 11               # quaternary bisection rounds: window 32/4^11
_DEBUG = False

_prog_cache = {}


def _build(debug=False):
    import concourse.bacc as bacc
    import concourse.mybir as mybir
    import concourse.tile as tile

    F32 = mybir.dt.float32
    F32R = mybir.dt.float32r
    AO = mybir.AluOpType
    AF = mybir.ActivationFunctionType

    nc = bacc.Bacc("TRN2", num_devices=N_CORES)

    tokTe_d = nc.dram_tensor("tokTe", [BPC, 8, 128, 4096], F32R,
                             kind="ExternalInput")
    xyzT_d = nc.dram_tensor("xyzT", [BPC, 3, N], F32R, kind="ExternalInput")
    xp_d = nc.dram_tensor("xp", [40, 2048], F32R, kind="ExternalInput")
    C5_d = nc.dram_tensor("C5", [40, 128], F32R, kind="ExternalInput")
    W1p_d = nc.dram_tensor("W1p", [515, 256], F32R, kind="ExternalInput")
    W2p_d = nc.dram_tensor("W2p", [128, 32], F32R, kind="ExternalInput")
    b1p_d = nc.dram_tensor("b1p", [128, 2], F32, kind="ExternalInput")
    b2bc_d = nc.dram_tensor("b2bc", [128, 1], F32, kind="ExternalInput")
    m2_d = nc.dram_tensor("m2", [128, 128], F32, kind="ExternalInput")
    mrep_d = nc.dram_tensor("mrep", [128, 128], F32R, kind="ExternalInput")

    disp_d = nc.dram_tensor("disp", [BPC, 128, 1024], F32, kind="ExternalOutput")
    comb_d = nc.dram_tensor("comb", [BPC, 128, 1024], F32, kind="ExternalOutput")
    if debug:
        dbg_logits_d = nc.dram_tensor("dbg_logits", [128, 2048], F32,
                                      kind="ExternalOutput")
        dbg_dists_d = nc.dram_tensor("dbg_dists", [128, 2048], F32,
                                     kind="ExternalOutput")

    with tile.TileContext(nc) as tc:
        with tc.tile_pool(name="const", bufs=1) as cpool, \
             tc.tile_pool(name="big", bufs=1) as bigpool, \
             tc.tile_pool(name="tok", bufs=2) as tokp, \
             tc.tile_pool(name="hb", bufs=4) as hb, \
             tc.tile_pool(name="work", bufs=2) as work, \
             tc.tile_pool(name="ps", bufs=1, space="PSUM") as ps, \
             tc.tile_pool(name="dram", bufs=1, space="DRAM") as dram:

            # ---- constants ----
            w1_sb = []
            for kc in range(5):
                kch = 128 if kc < 4 else 3
                t = cpool.tile([kch, 256], F32R, tag=f"w1_{kc}", name=f"w1_{kc}")
                nc.sync.dma_start(out=t[:], in_=W1p_d[kc * 128:kc * 128 + kch, :])
                w1_sb.append(t)
            w2_sb = cpool.tile([128, 32], F32R, tag="w2", name="w2")
            nc.sync.dma_start(out=w2_sb[:], in_=W2p_d[:])
            b1_sb = cpool.tile([128, 2], F32, tag="b1", name="b1")
            nc.sync.dma_start(out=b1_sb[:], in_=b1p_d[:])
            b2bc_sb = cpool.tile([128, 1], F32, tag="b2bc", name="b2bc")
            nc.sync.dma_start(out=b2bc_sb[:], in_=b2bc_d[:])
            C5_sb = cpool.tile([40, 128], F32R, tag="C5", name="C5")
            nc.sync.dma_start(out=C5_sb[:], in_=C5_d[:])
            xp_sb = cpool.tile([40, 2048], F32R, tag="xp", name="xp")
            nc.sync.dma_start(out=xp_sb[:], in_=xp_d[:])
            m2_sb = cpool.tile([128, 128], F32, tag="m2", name="m2")
            nc.sync.dma_start(out=m2_sb[:], in_=m2_d[:])
            mrep_sb = cpool.tile([128, 128], F32R, tag="mrep", name="mrep")
            nc.sync.dma_start(out=mrep_sb[:], in_=mrep_d[:])
            ones_1x128 = cpool.tile([1, 128], F32, tag="o1x", name="o1x")
            nc.vector.memset(ones_1x128[:], 1.0)
            ones_128x1 = cpool.tile([128, 1], F32, tag="ox1", name="ox1")
            nc.vector.memset(ones_128x1[:], 1.0)

            # ---- persistent tiles ----
            logits_A = bigpool.tile([128, 2048], F32, tag="logits", name="logits")
            dists_A = bigpool.tile([128, 2048], F32, tag="dists", name="dists")
            sig_A = bigpool.tile([128, 2048], F32, tag="sig", name="sig")

            # ============ dists + global mean (both batches early) ==========
            for b in range(BPC):
                for blk in range(2):
                    i = 2 * b + blk
                    p_d = ps.tile([128, 512], F32, tag="ph", name="p_d", bufs=3)
                    nc.tensor.matmul(p_d[:], C5_sb[:],
                                     xp_sb[:, i * 512:(i + 1) * 512],
                                     start=True, stop=True)
                    off = b * 1024 + blk * 512
                    nc.scalar.activation(dists_A[:, off:off + 512], p_d[:],
                                         AF.Sqrt)

            rsum = work.tile([128, 1], F32, tag="rsum", name="rsum")
            nc.vector.tensor_reduce(out=rsum[:], in_=dists_A[:],
                                    axis=mybir.AxisListType.X, op=AO.add)
            p_tot = ps.tile([1, 1], F32, tag="pcnt", name="p_tot", bufs=1)
            nc.tensor.matmul(p_tot[:], ones_128x1[:], rsum[:], start=True, stop=True)
            s_tot = work.tile([1, 1], F32, tag="stot", name="stot")
            nc.vector.tensor_copy(s_tot[:], p_tot[:])
            p_bc = ps.tile([128, 1], F32, tag="pcnt", name="p_bc", bufs=1)
            nc.tensor.matmul(p_bc[:], ones_1x128[:], s_tot[:], start=True, stop=True)
            sb_bc = work.tile([128, 1], F32, tag="sbbc", name="sbbc")
            nc.vector.tensor_copy(sb_bc[:], p_bc[:])
            cc_in = dram.tile([128, 1], F32)
            cc_out = dram.tile([128, 1], F32, addr_space="Shared")
            nc.sync.dma_start(out=cc_in[:], in_=sb_bc[:])
            nc.gpsimd.collective_compute(
                "AllReduce", AO.add, ins=[cc_in.opt()], outs=[cc_out.opt()],
                replica_groups=[list(range(N_CORES))])
            S_sb = bigpool.tile([128, 1], F32, tag="S", name="S")
            nc.sync.dma_start(out=S_sb[:], in_=cc_out[:])
            m_sb = bigpool.tile([128, 1], F32, tag="m", name="m")
            nc.vector.tensor_scalar(out=m_sb[:], in0=S_sb[:],
                                    scalar1=1.0 / (B * N * E), scalar2=1e-6,
                                    op0=AO.mult, op1=AO.add)
            r_sb = bigpool.tile([128, 1], F32, tag="r", name="r")
            nc.vector.reciprocal(r_sb[:], m_sb[:])
            a_sb = bigpool.tile([128, 1], F32, tag="a", name="a")
            nc.vector.tensor_scalar(out=a_sb[:], in0=r_sb[:], scalar1=-1.0,
                                    scalar2=None, op0=AO.mult)

            # ============ gate MLP =========================================
            def mlp_batch(b):
                # xyz rows for the whole batch in one DMA
                xb = tokp.tile([3, N], F32R, tag="xb", name="xb", bufs=2)
                nc.sync.dma_start(out=xb[:], in_=xyzT_d[b])
                # eighths of 1024 tokens; host layout is eighth-contiguous so
                # each partition line is one 16KB descriptor run
                for q in range(8):
                    tk = tokp.tile([128, 4096], F32R, tag="tk", name="tk")
                    nc.sync.dma_start(out=tk[:], in_=tokTe_d[b, q])
                    for s in range(2):
                        T = 2 * q + s            # chunk id: blk = T//8, g = T%8
                        h_sb = []
                        for mc in range(2):
                            p_h = ps.tile([128, 512], F32, tag="ph",
                                          name="p_h", bufs=3)
                            for kc in range(5):
                                rhs = (tk[:, kc * 1024 + s * 512:
                                          kc * 1024 + (s + 1) * 512]
                                       if kc < 4 else
                                       xb[:, q * 1024 + s * 512:
                                          q * 1024 + (s + 1) * 512])
                                nc.tensor.matmul(
                                    p_h[:],
                                    w1_sb[kc][:, mc * 128:(mc + 1) * 128],
                                    rhs,
                                    start=(kc == 0), stop=(kc == 4))
                            t_h = hb.tile([128, 512], F32R, tag=f"h{mc}",
                                          name=f"h{mc}")
                            nc.scalar.activation(t_h[:], p_h[:], AF.Gelu,
                                                 bias=b1_sb[:, mc:mc + 1],
                                                 scale=1.0)
                            h_sb.append(t_h)
                        p_l2 = ps.tile([16, 512], F32, tag="pl2",
                                       name="p_l2", bufs=2)
                        for mc in range(2):
                            nc.tensor.matmul(p_l2[:],
                                             w2_sb[:, mc * 16:(mc + 1) * 16],
                                             h_sb[mc][:],
                                             start=(mc == 0), stop=(mc == 1))
                        t_st = hb.tile([16, 512], F32, tag="tst", name="tst",
                                       bufs=3)
                        nc.scalar.activation(t_st[:], p_l2[:], AF.Copy)
                        blk, g = T // 8, T % 8
                        nc.sync.dma_start(
                            out=logits_A[16 * g:16 * (g + 1),
                                         b * 1024 + blk * 512:
                                         b * 1024 + blk * 512 + 512],
                            in_=t_st[:])

            def finalize(b):
                sl = slice(b * 1024, (b + 1) * 1024)
                nc.vector.scalar_tensor_tensor(
                    out=logits_A[:, sl], in0=dists_A[:, sl], scalar=a_sb[:],
                    in1=logits_A[:, sl], op0=AO.mult, op1=AO.add)
                nc.vector.tensor_scalar(out=logits_A[:, sl], in0=logits_A[:, sl],
                                        scalar1=b2bc_sb[:], scalar2=None,
                                        op0=AO.add)
                nc.scalar.activation(sig_A[:, sl], logits_A[:, sl], AF.Sigmoid)

            # quaternary bisection for the top-KSEL threshold per (b,e)
            theta = [bigpool.tile([128, 1], F32, tag=f"th{b}", name=f"th{b}")
                     for b in range(BPC)]

            def bisect(b):
                sl = slice(b * 1024, (b + 1) * 1024)
                mu = theta[b]
                nc.vector.memset(mu[:], 0.0)
                msk = work.tile([128, 1024], F32, tag=f"msk{b}",
                                name=f"msk{b}", bufs=1)
                cj = work.tile([128, 3], F32, tag=f"cj{b}", name=f"cj{b}",
                               bufs=2)
                acc = work.tile([128, 3], F32, tag=f"ac{b}", name=f"ac{b}",
                                bufs=2)
                s3 = work.tile([128, 3], F32, tag=f"s3{b}", name=f"s3{b}",
                               bufs=2)
                sig1 = work.tile([128, 1], F32, tag=f"sg{b}", name=f"sg{b}",
                                 bufs=2)
                K = 0.0
                Wd = 32.0
                for r in range(N_ROUNDS):
                    for j in range(3):
                        nc.vector.tensor_scalar(
                            out=cj[:, j:j + 1], in0=mu[:],
                            scalar1=K + (j - 1) * Wd / 4, scalar2=None,
                            op0=AO.add)
                        nc.vector.tensor_scalar(
                            out=msk[:], in0=logits_A[:, sl],
                            scalar1=cj[:, j:j + 1], scalar2=0.0,
                            op0=AO.is_gt, op1=AO.add,
                            accum_out=acc[:, j:j + 1])
                    p_cnt = ps.tile([128, 3], F32, tag="pcnt", name="p_cnt",
                                    bufs=1)
                    nc.tensor.matmul(p_cnt[:], m2_sb[:], acc[:],
                                     start=True, stop=True)
                    nc.vector.tensor_scalar(
                        out=s3[:], in0=p_cnt[:], scalar1=float(KSEL),
                        scalar2=0.0, op0=AO.is_ge, op1=AO.add,
                        accum_out=sig1[:])
                    nc.vector.scalar_tensor_tensor(
                        out=mu[:], in0=sig1[:], scalar=Wd / 4, in1=mu[:],
                        op0=AO.mult, op1=AO.add)
                    K = K - 0.375 * Wd
                    Wd = Wd / 4
                # theta := left edge of the final interval
                nc.vector.tensor_scalar(out=mu[:], in0=mu[:],
                                        scalar1=K - Wd / 2, scalar2=None,
                                        op0=AO.add)

            def emit(b):
                sl = slice(b * 1024, (b + 1) * 1024)
                # reuse the bisect mask buffer as dispM scratch
                dispM = work.tile([128, 1024], F32, tag=f"msk{b}",
                                  name=f"dm{b}", bufs=1)
                nc.vector.scalar_tensor_tensor(
                    out=dispM[:], in0=logits_A[:, sl], scalar=theta[b][:],
                    in1=sig_A[:, sl], op0=AO.is_gt, op1=AO.mult)
                dispO = work.tile([128, 1024], F32R, tag=f"do{b}",
                                  name=f"do{b}", bufs=1)
                nc.vector.tensor_scalar(out=dispO[:], in0=dispM[:],
                                        scalar1=DSCALE, scalar2=DFLOOR,
                                        op0=AO.mult, op1=AO.add)
                nc.sync.dma_start(out=disp_d[b], in_=dispO[:].bitcast(F32))
                # combine = disp / rowsum_e(disp)
                comb = work.tile([128, 1024], F32, tag=f"cb{b}", name=f"cb{b}",
                                 bufs=1)
                den = work.tile([128, 1024], F32, tag=f"dn{b}", name=f"dn{b}",
                                bufs=1)
                for hhalf in range(2):
                    hsl = slice(hhalf * 512, (hhalf + 1) * 512)
                    p_rep = ps.tile([128, 512], F32, tag="ph", name="p_rep",
                                    bufs=3)
                    nc.tensor.matmul(p_rep[:], mrep_sb[:], dispO[:, hsl],
                                     start=True, stop=True)
                    nc.vector.reciprocal(den[:, hsl], p_rep[:])
                    nc.vector.tensor_tensor(out=comb[:, hsl],
                                            in0=dispO[:, hsl].bitcast(F32),
                                            in1=den[:, hsl], op=AO.mult)
                nc.sync.dma_start(out=comb_d[b], in_=comb[:])

            mlp_batch(0)
            mlp_batch(1)
            finalize(0)
            bisect(0)
            emit(0)
            finalize(1)
            bisect(1)
            emit(1)
            if debug:
                nc.sync.dma_start(out=dbg_dists_d[:], in_=dists_A[:])
                nc.sync.dma_start(out=dbg_logits_d[:], in_=logits_A[:])

    nc.finalize()
    return nc


def _get_prog(debug=False):
    key = ("prog", debug)
    if key not in _prog_cache:
        _prog_cache[key] = _build(debug)
    return _prog_cache[key]


def make_in_maps(inputs):
    tokens = np.asarray(inputs["tokens"], dtype=np.float32)
    xyz = np.asarray(inputs["spatial_xyz"], dtype=np.float32)
    W1 = np.ascontiguousarray(np.asarray(inputs["W1"], dtype=np.float32))
    b1 = np.asarray(inputs["b1"], dtype=np.float32)
    W2 = np.asarray(inputs["W2"], dtype=np.float32)
    b2 = np.asarray(inputs["b2"], dtype=np.float32)
    centers = np.asarray(inputs["centers"], dtype=np.float32)

    # shared constants
    W2p = np.zeros((128, 32), np.float32)
    for mc in range(2):
        W2p[:, mc * 16:(mc + 1) * 16] = W2[mc * 128:(mc + 1) * 128, :]
    b1p = np.ascontiguousarray(b1.reshape(2, 128).T)
    b2bc = np.ascontiguousarray(np.tile(b2, 8)[:, None].astype(np.float32))
    m2 = np.ascontiguousarray(
        (np.arange(128)[:, None] % 16 == np.arange(128)[None, :] % 16)
        .astype(np.float32))
    mrep = np.ascontiguousarray(
        (np.arange(128)[:, None] // 16 == np.arange(128)[None, :] // 16)
        .astype(np.float32))
    # C5 [40, 128]: col 16g+e, rows 5g+c = [-2cx, -2cy, -2cz, 1, |c|^2]
    C5 = np.zeros((40, 128), np.float32)
    csq = (centers.astype(np.float64) ** 2).sum(-1).astype(np.float32)
    for g in range(8):
        for e in range(E):
            p = 16 * g + e
            C5[5 * g + 0, p] = -2.0 * centers[e, 0]
            C5[5 * g + 1, p] = -2.0 * centers[e, 1]
            C5[5 * g + 2, p] = -2.0 * centers[e, 2]
            C5[5 * g + 3, p] = 1.0
            C5[5 * g + 4, p] = csq[e]

    in_maps = []
    for c in range(N_CORES):
        tokTe = np.empty((BPC, 8, 128, 4096), np.float32)
        xyzT = np.empty((BPC, 3, N), np.float32)
        xp = np.empty((40, 2048), np.float32)
        for b in range(BPC):
            gb = BPC * c + b
            # tokTe[b, q, p, kc*1024+c] = tokens[gb, q*1024+c, kc*128+p]
            tokTe[b] = (tokens[gb].reshape(8, 1024, 4, 128)
                        .transpose(0, 3, 2, 1).reshape(8, 128, 4096))
            xyzT[b] = xyz[gb].T
            x2 = (xyz[gb].astype(np.float64) ** 2).sum(-1).astype(np.float32)
            for blk in range(2):
                i = 2 * b + blk
                # rows 5g+c over tokens (g, t) of this blk
                xs = xyz[gb][(blk * 8) * 512:(blk * 8 + 8) * 512]  # (8*512, 3)
                xs = xs.reshape(8, 512, 3)
                x2s = x2[(blk * 8) * 512:(blk * 8 + 8) * 512].reshape(8, 512)
                blkm = np.empty((8, 5, 512), np.float32)
                blkm[:, 0:3] = xs.transpose(0, 2, 1)
                blkm[:, 3] = x2s
                blkm[:, 4] = 1.0
                xp[:, i * 512:(i + 1) * 512] = blkm.reshape(40, 512)
        in_maps.append({
            "tokTe": np.ascontiguousarray(tokTe),
            "xyzT": np.ascontiguousarray(xyzT),
            "xp": np.ascontiguousarray(xp),
            "C5": C5, "W1p": W1, "W2p": W2p, "b1p": b1p, "b2bc": b2bc,
            "m2": m2, "mrep": mrep,
        })
    return in_maps


def _unlayout(arr):
    """[128, 1024] (p=16g+e, f=blk*512+t) -> (8192, 16)"""
    return (arr.reshape(8, 16, 2, 512).transpose(2, 0, 3, 1)
            .reshape(N, E))


def kernel(**inputs):
    from concourse.bass_utils import run_bass_kernel_spmd

    nc = _get_prog(_DEBUG)
    in_maps = make_in_maps(inputs)
    res = run_bass_kernel_spmd(nc, in_maps, list(range(N_CORES)))
    dispatch = np.empty((B, N, E), np.float32)
    combine = np.empty((B, N, E), np.float32)
    for c in range(N_CORES):
        for b in range(BPC):
            dispatch[BPC * c + b] = _unlayout(res.results[c]["disp"][b])
            combine[BPC * c + b] = _unlayout(res.results[c]["comb"][b])
    if _DEBUG:
        kernel._dbg = [(res.results[c]["dbg_logits"], res.results[c]["dbg_dists"])
                       for c in range(N_CORES)]
    return dispatch, combine


# revision 29
# speedup vs baseline: 1.2092x; 1.2092x over previous
"""Trainium2 Bass kernel for nn_ModalityMoERouter (expert-choice MoE routing).

Contract: kernel(**inputs) takes the FULL inputs from reference.setup_inputs()
and returns (dispatch, combine), each (16, 8192, 16) float32.

Sharding: data-parallel over batch B=16 across 8 NeuronCores (2 batches/core);
gate weights and expert centers replicated. The global mean(dists) scalar is
computed with one AllReduce (overlapped with the gate MLP).

Key design points vs the fp32 baseline (667 us):
 - All large matmuls run in float32r (1 cycle/row on TRN2 vs 4 for fp32;
   ~11-bit mantissa, rel err ~1.5e-4 -- measured on HW; final output rel err
   ~1.4e-2, within the 2e-2 gate).
 - Tokens are transposed on the HOST (numpy) to [D, N] layout, so the kernel
   needs zero PE transposes and no PSUM-evacuation copies for activations.
 - dists(token, e) come from ONE small fp32r matmul per 512-token block using
   rows [x, y, z, |x|^2, 1] against columns [-2cx, -2cy, -2cz, 1, |c|^2];
   sqrt on ACT with accum_out gives the global-mean numerator for free.
 - Expert-choice top-k threshold via quaternary bisection (3 compares/round,
   11 rounds) using tensor_scalar is_gt + accum_out (2x_2P DVE mode).
 - Outputs are written in the on-chip (g,e)-partition layout and untransposed
   on the host.

On-chip layout: [128, 1024] per batch with partition p = g*16+e and free
f = blk*512 + t, where token n = (blk*8+g)*512 + t.
The hard-cap step of the reference is a bitwise no-op (dispatch <= 0.4375 <
cap >= 0.5), so it is skipped (t unused).
"""

import numpy as np

B = 16
N = 8192
D = 512
H = 256
E = 16
N_CORES = 8
BPC = B // N_CORES
KSEL = N * 2 // E           # 1024
ALPHA = min(min(0.05, 0.15 / 4) * E, 1.0)
DSCALE = 1.0 - ALPHA        # 0.4
DFLOOR = ALPHA / E          # 0.0375
N_ROUNDS = 11               # quaternary bisection rounds: window 32/4^11
_DEBUG = False

_prog_cache = {}


def _build(debug=False):
    import concourse.bacc as bacc
    import concourse.mybir as mybir
    import concourse.tile as tile

    F32 = mybir.dt.float32
    F32R = mybir.dt.float32r
    AO = mybir.AluOpType
    AF = mybir.ActivationFunctionType

    nc = bacc.Bacc("TRN2", num_devices=N_CORES)

    tokTe_d = nc.dram_tensor("tokTe", [BPC, 8, 128, 4096], F32R,
                             kind="ExternalInput")
    xyzT_d = nc.dram_tensor("xyzT", [BPC, 3, N], F32R, kind="ExternalInput")
    xp_d = nc.dram_tensor("xp", [40, 2048], F32R, kind="ExternalInput")
    C5_d = nc.dram_tensor("C5", [40, 128], F32R, kind="ExternalInput")
    W1p_d = nc.dram_tensor("W1p", [515, 256], F32R, kind="ExternalInput")
    W2p_d = nc.dram_tensor("W2p", [128, 32], F32R, kind="ExternalInput")
    b1p_d = nc.dram_tensor("b1p", [128, 2], F32, kind="ExternalInput")
    b2bc_d = nc.dram_tensor("b2bc", [128, 1], F32, kind="ExternalInput")
    m2_d = nc.dram_tensor("m2", [128, 128], F32, kind="ExternalInput")
    mrep_d = nc.dram_tensor("mrep", [128, 128], F32R, kind="ExternalInput")

    disp_d = nc.dram_tensor("disp", [BPC, 128, 1024], F32, kind="ExternalOutput")
    comb_d = nc.dram_tensor("comb", [BPC, 128, 1024], F32, kind="ExternalOutput")
    if debug:
        dbg_logits_d = nc.dram_tensor("dbg_logits", [128, 2048], F32,
                                      kind="ExternalOutput")
        dbg_dists_d = nc.dram_tensor("dbg_dists", [128, 2048], F32,
                                     kind="ExternalOutput")

    with tile.TileContext(nc) as tc:
        with tc.tile_pool(name="const", bufs=1) as cpool, \
             tc.tile_pool(name="big", bufs=1) as bigpool, \
             tc.tile_pool(name="tok", bufs=2) as tokp, \
             tc.tile_pool(name="hb", bufs=4) as hb, \
             tc.tile_pool(name="work", bufs=2) as work, \
             tc.tile_pool(name="ps", bufs=1, space="PSUM") as ps, \
             tc.tile_pool(name="dram", bufs=1, space="DRAM") as dram:

            # ---- constants ----
            w1_sb = []
            for kc in range(5):
                kch = 128 if kc < 4 else 3
                t = cpool.tile([kch, 256], F32R, tag=f"w1_{kc}", name=f"w1_{kc}")
                nc.sync.dma_start(out=t[:], in_=W1p_d[kc * 128:kc * 128 + kch, :])
                w1_sb.append(t)
            w2_sb = cpool.tile([128, 32], F32R, tag="w2", name="w2")
            nc.sync.dma_start(out=w2_sb[:], in_=W2p_d[:])
            b1_sb = cpool.tile([128, 2], F32, tag="b1", name="b1")
            nc.sync.dma_start(out=b1_sb[:], in_=b1p_d[:])
            b2bc_sb = cpool.tile([128, 1], F32, tag="b2bc", name="b2bc")
            nc.sync.dma_start(out=b2bc_sb[:], in_=b2bc_d[:])
            C5_sb = cpool.tile([40, 128], F32R, tag="C5", name="C5")
            nc.sync.dma_start(out=C5_sb[:], in_=C5_d[:])
            xp_sb = cpool.tile([40, 2048], F32R, tag="xp", name="xp")
            nc.sync.dma_start(out=xp_sb[:], in_=xp_d[:])
            m2_sb = cpool.tile([128, 128], F32, tag="m2", name="m2")
            nc.sync.dma_start(out=m2_sb[:], in_=m2_d[:])
            mrep_sb = cpool.tile([128, 128], F32R, tag="mrep", name="mrep")
            nc.sync.dma_start(out=mrep_sb[:], in_=mrep_d[:])
            ones_1x128 = cpool.tile([1, 128], F32, tag="o1x", name="o1x")
            nc.vector.memset(ones_1x128[:], 1.0)
            ones_128x1 = cpool.tile([128, 1], F32, tag="ox1", name="ox1")
            nc.vector.memset(ones_128x1[:], 1.0)

            # ---- persistent tiles ----
            logits_A = bigpool.tile([128, 2048], F32, tag="logits", name="logits")
            dists_A = bigpool.tile([128, 2048], F32, tag="dists", name="dists")
            sig_A = bigpool.tile([128, 2048], F32, tag="sig", name="sig")

            # ============ dists + global mean (both batches early) ==========
            for b in range(BPC):
                for blk in range(2):
                    i = 2 * b + blk
                    p_d = ps.tile([128, 512], F32, tag="ph", name="p_d", bufs=4)
                    nc.tensor.matmul(p_d[:], C5_sb[:],
                                     xp_sb[:, i * 512:(i + 1) * 512],
                                     start=True, stop=True)
                    off = b * 1024 + blk * 512
                    nc.scalar.activation(dists_A[:, off:off + 512], p_d[:],
                                         AF.Sqrt)

            rsum = work.tile([128, 1], F32, tag="rsum", name="rsum")
            nc.vector.tensor_reduce(out=rsum[:], in_=dists_A[:],
                                    axis=mybir.AxisListType.X, op=AO.add)
            p_tot = ps.tile([1, 1], F32, tag="pcnt", name="p_tot", bufs=1)
            nc.tensor.matmul(p_tot[:], ones_128x1[:], rsum[:], start=True, stop=True)
            s_tot = work.tile([1, 1], F32, tag="stot", name="stot")
            nc.vector.tensor_copy(s_tot[:], p_tot[:])
            p_bc = ps.tile([128, 1], F32, tag="pcnt", name="p_bc", bufs=1)
            nc.tensor.matmul(p_bc[:], ones_1x128[:], s_tot[:], start=True, stop=True)
            sb_bc = work.tile([128, 1], F32, tag="sbbc", name="sbbc")
            nc.vector.tensor_copy(sb_bc[:], p_bc[:])
            cc_in = dram.tile([128, 1], F32)
            cc_out = dram.tile([128, 1], F32, addr_space="Shared")
            nc.sync.dma_start(out=cc_in[:], in_=sb_bc[:])
            nc.gpsimd.collective_compute(
                "AllReduce", AO.add, ins=[cc_in.opt()], outs=[cc_out.opt()],
                replica_groups=[list(range(N_CORES))])
            S_sb = bigpool.tile([128, 1], F32, tag="S", name="S")
            nc.sync.dma_start(out=S_sb[:], in_=cc_out[:])
            m_sb = bigpool.tile([128, 1], F32, tag="m", name="m")
            nc.vector.tensor_scalar(out=m_sb[:], in0=S_sb[:],
                                    scalar1=1.0 / (B * N * E), scalar2=1e-6,
                                    op0=AO.mult, op1=AO.add)
            r_sb = bigpool.tile([128, 1], F32, tag="r", name="r")
            nc.vector.reciprocal(r_sb[:], m_sb[:])
            a_sb = bigpool.tile([128, 1], F32, tag="a", name="a")
            nc.vector.tensor_scalar(out=a_sb[:], in0=r_sb[:], scalar1=-1.0,
                                    scalar2=None, op0=AO.mult)

            # ============ gate MLP =========================================
            def emit_w2(b, T, h_sb):
                p_l2 = ps.tile([16, 512], F32, tag="pl2", name="p_l2", bufs=3)
                for mc in range(2):
                    nc.tensor.matmul(p_l2[:],
                                     w2_sb[:, mc * 16:(mc + 1) * 16],
                                     h_sb[mc][:],
                                     start=(mc == 0), stop=(mc == 1))
                t_st = hb.tile([16, 512], F32, tag="tst", name="tst", bufs=6)
                nc.scalar.activation(t_st[:], p_l2[:], AF.Copy)
                blk, g = T // 8, T % 8
                nc.gpsimd.dma_start(
                    out=logits_A[16 * g:16 * (g + 1),
                                 b * 1024 + blk * 512:
                                 b * 1024 + blk * 512 + 512],
                    in_=t_st[:])

            def mlp_batch(b):
                # xyz rows for the whole batch in one DMA
                xb = tokp.tile([3, N], F32R, tag="xb", name="xb", bufs=2)
                nc.sync.dma_start(out=xb[:], in_=xyzT_d[b])
                # eighths of 1024 tokens; host layout is eighth-contiguous so
                # each partition line is one 16KB descriptor run
                prev = None
                for q in range(8):
                    tk = tokp.tile([128, 4096], F32R, tag="tk", name="tk")
                    nc.sync.dma_start(out=tk[:], in_=tokTe_d[b, q])
                    for s in range(2):
                        T = 2 * q + s            # chunk id: blk = T//8, g = T%8
                        h_sb = []
                        for mc in range(2):
                            p_h = ps.tile([128, 512], F32, tag="ph",
                                          name="p_h", bufs=4)
                            for kc in range(5):
                                rhs = (tk[:, kc * 1024 + s * 512:
                                          kc * 1024 + (s + 1) * 512]
                                       if kc < 4 else
                                       xb[:, q * 1024 + s * 512:
                                          q * 1024 + (s + 1) * 512])
                                nc.tensor.matmul(
                                    p_h[:],
                                    w1_sb[kc][:, mc * 128:(mc + 1) * 128],
                                    rhs,
                                    start=(kc == 0), stop=(kc == 4))
                            t_h = hb.tile([128, 512], F32R, tag=f"h{mc}",
                                          name=f"h{mc}")
                            nc.scalar.activation(t_h[:], p_h[:], AF.Gelu,
                                                 bias=b1_sb[:, mc:mc + 1],
                                                 scale=1.0)
                            h_sb.append(t_h)
                        # W2 stage runs one chunk behind so the PE never
                        # waits on the gelu of the chunk it just produced
                        if prev is not None:
                            emit_w2(b, prev[0], prev[1])
                        prev = (T, h_sb)
                emit_w2(b, prev[0], prev[1])

            def finalize(b):
                sl = slice(b * 1024, (b + 1) * 1024)
                nc.vector.scalar_tensor_tensor(
                    out=logits_A[:, sl], in0=dists_A[:, sl], scalar=a_sb[:],
                    in1=logits_A[:, sl], op0=AO.mult, op1=AO.add)
                nc.vector.tensor_scalar(out=logits_A[:, sl], in0=logits_A[:, sl],
                                        scalar1=b2bc_sb[:], scalar2=None,
                                        op0=AO.add)
                nc.scalar.activation(sig_A[:, sl], logits_A[:, sl], AF.Sigmoid)

            # quaternary bisection for the top-KSEL threshold per (b,e)
            theta = [bigpool.tile([128, 1], F32, tag=f"th{b}", name=f"th{b}")
                     for b in range(BPC)]

            def bisect(b):
                # Quaternary bisection; per round the 3 interval-edge counts
                # come from 1 DVE is_gt+accum and 2 ACT Sign+accum compares
                # running concurrently. Units: DVE col gives cnt-512/lane,
                # ACT cols give 2*cnt-1024/lane; after the m2 partition-sum
                # the is_ge thresholds are KSEL-4096 and 2*KSEL-8192.
                sl = slice(b * 1024, (b + 1) * 1024)
                mu = theta[b]
                nc.vector.memset(mu[:], 0.0)
                msk = work.tile([128, 1024], F32, tag=f"msk{b}",
                                name=f"msk{b}", bufs=1)
                mskA = [work.tile([128, 1024], F32, tag=f"mskA{j}",
                                  name=f"mskA{b}{j}", bufs=1)
                        for j in range(2)]
                cj = work.tile([128, 3], F32, tag=f"cj{b}", name=f"cj{b}",
                               bufs=2)
                acc = work.tile([128, 3], F32, tag=f"ac{b}", name=f"ac{b}",
                                bufs=2)
                s1 = work.tile([128, 1], F32, tag=f"s1{b}", name=f"s1{b}",
                               bufs=2)
                s2d = work.tile([128, 2], F32, tag=f"s2{b}", name=f"s2{b}",
                                bufs=2)
                sig2 = work.tile([128, 1], F32, tag=f"sg{b}", name=f"sg{b}",
                                 bufs=2)
                K = 0.0
                Wd = 32.0
                for r in range(N_ROUNDS):
                    # c_0 = mu + K - W/4 (DVE); nb_j = -(mu + K + (j-1)W/4)
                    nc.vector.tensor_scalar(
                        out=cj[:, 0:1], in0=mu[:],
                        scalar1=K - Wd / 4, scalar2=None, op0=AO.add)
                    for j in (1, 2):
                        nc.vector.tensor_scalar(
                            out=cj[:, j:j + 1], in0=mu[:],
                            scalar1=K + (j - 1) * Wd / 4, scalar2=-1.0,
                            op0=AO.add, op1=AO.mult)
                    nc.vector.tensor_scalar(
                        out=msk[:], in0=logits_A[:, sl],
                        scalar1=cj[:, 0:1], scalar2=0.0,
                        op0=AO.is_gt, op1=AO.add,
                        accum_out=acc[:, 0:1])
                    for j in (1, 2):
                        nc.scalar.activation(
                            mskA[j - 1][:], logits_A[:, sl], AF.Sign,
                            bias=cj[:, j:j + 1], scale=1.0,
                            accum_out=acc[:, j:j + 1])
                    p_cnt = ps.tile([128, 3], F32, tag="pcnt", name="p_cnt",
                                    bufs=1)
                    nc.tensor.matmul(p_cnt[:], m2_sb[:], acc[:],
                                     start=True, stop=True)
                    nc.vector.tensor_scalar(
                        out=s1[:], in0=p_cnt[:, 0:1],
                        scalar1=float(KSEL), scalar2=None, op0=AO.is_ge)
                    nc.vector.tensor_scalar(
                        out=s2d[:], in0=p_cnt[:, 1:3],
                        scalar1=float(2 * KSEL - 8192), scalar2=0.0,
                        op0=AO.is_ge, op1=AO.add, accum_out=sig2[:])
                    nc.vector.scalar_tensor_tensor(
                        out=mu[:], in0=s1[:], scalar=Wd / 4, in1=mu[:],
                        op0=AO.mult, op1=AO.add)
                    nc.vector.scalar_tensor_tensor(
                        out=mu[:], in0=sig2[:], scalar=Wd / 4, in1=mu[:],
                        op0=AO.mult, op1=AO.add)
                    K = K - 0.375 * Wd
                    Wd = Wd / 4
                # theta := left edge of the final interval
                nc.vector.tensor_scalar(out=mu[:], in0=mu[:],
                                        scalar1=K - Wd / 2, scalar2=None,
                                        op0=AO.add)

            def emit(b):
                sl = slice(b * 1024, (b + 1) * 1024)
                # reuse the bisect mask buffer as dispM scratch
                dispM = work.tile([128, 1024], F32, tag=f"msk{b}",
                                  name=f"dm{b}", bufs=1)
                nc.vector.scalar_tensor_tensor(
                    out=dispM[:], in0=logits_A[:, sl], scalar=theta[b][:],
                    in1=sig_A[:, sl], op0=AO.is_gt, op1=AO.mult)
                dispO = work.tile([128, 1024], F32R, tag=f"do{b}",
                                  name=f"do{b}", bufs=1)
                nc.vector.tensor_scalar(out=dispO[:], in0=dispM[:],
                                        scalar1=DSCALE, scalar2=DFLOOR,
                                        op0=AO.mult, op1=AO.add)
                nc.sync.dma_start(out=disp_d[b], in_=dispO[:].bitcast(F32))
                # combine = disp / rowsum_e(disp)
                comb = work.tile([128, 1024], F32, tag=f"cb{b}", name=f"cb{b}",
                                 bufs=1)
                den = work.tile([128, 1024], F32, tag=f"dn{b}", name=f"dn{b}",
                                bufs=1)
                for hhalf in range(2):
                    hsl = slice(hhalf * 512, (hhalf + 1) * 512)
                    p_rep = ps.tile([128, 512], F32, tag="ph", name="p_rep",
                                    bufs=4)
                    nc.tensor.matmul(p_rep[:], mrep_sb[:], dispO[:, hsl],
                                     start=True, stop=True)
                    nc.vector.reciprocal_approx_fast(out=den[:, hsl],
                                                     in_=p_rep[:])
                    nc.vector.tensor_tensor(out=comb[:, hsl],
                                            in0=dispO[:, hsl].bitcast(F32),
                                            in1=den[:, hsl], op=AO.mult)
                nc.sync.dma_start(out=comb_d[b], in_=comb[:])

            mlp_batch(0)
            mlp_batch(1)
            finalize(0)
            bisect(0)
            emit(0)
            finalize(1)
            bisect(1)
            emit(1)
            if debug:
                nc.sync.dma_start(out=dbg_dists_d[:], in_=dists_A[:])
                nc.sync.dma_start(out=dbg_logits_d[:], in_=logits_A[:])

    nc.finalize()
    return nc


def _get_prog(debug=False):
    key = ("prog", debug)
    if key not in _prog_cache:
        _prog_cache[key] = _build(debug)
    return _prog_cache[key]


def make_in_maps(inputs):
    tokens = np.asarray(inputs["tokens"], dtype=np.float32)
    xyz = np.asarray(inputs["spatial_xyz"], dtype=np.float32)
    W1 = np.ascontiguousarray(np.asarray(inputs["W1"], dtype=np.float32))
    b1 = np.asarray(inputs["b1"], dtype=np.float32)
    W2 = np.asarray(inputs["W2"], dtype=np.float32)
    b2 = np.asarray(inputs["b2"], dtype=np.float32)
    centers = np.asarray(inputs["centers"], dtype=np.float32)

    # shared constants
    W2p = np.zeros((128, 32), np.float32)
    for mc in range(2):
        W2p[:, mc * 16:(mc + 1) * 16] = W2[mc * 128:(mc + 1) * 128, :]
    b1p = np.ascontiguousarray(b1.reshape(2, 128).T)
    b2bc = np.ascontiguousarray(np.tile(b2, 8)[:, None].astype(np.float32))
    m2 = np.ascontiguousarray(
        (np.arange(128)[:, None] % 16 == np.arange(128)[None, :] % 16)
        .astype(np.float32))
    mrep = np.ascontiguousarray(
        (np.arange(128)[:, None] // 16 == np.arange(128)[None, :] // 16)
        .astype(np.float32))
    # C5 [40, 128]: col 16g+e, rows 5g+c = [-2cx, -2cy, -2cz, 1, |c|^2]
    C5 = np.zeros((40, 128), np.float32)
    csq = (centers.astype(np.float64) ** 2).sum(-1).astype(np.float32)
    for g in range(8):
        for e in range(E):
            p = 16 * g + e
            C5[5 * g + 0, p] = -2.0 * centers[e, 0]
            C5[5 * g + 1, p] = -2.0 * centers[e, 1]
            C5[5 * g + 2, p] = -2.0 * centers[e, 2]
            C5[5 * g + 3, p] = 1.0
            C5[5 * g + 4, p] = csq[e]

    in_maps = []
    for c in range(N_CORES):
        tokTe = np.empty((BPC, 8, 128, 4096), np.float32)
        xyzT = np.empty((BPC, 3, N), np.float32)
        xp = np.empty((40, 2048), np.float32)
        for b in range(BPC):
            gb = BPC * c + b
            # tokTe[b, q, p, kc*1024+c] = tokens[gb, q*1024+c, kc*128+p]
            tokTe[b] = (tokens[gb].reshape(8, 1024, 4, 128)
                        .transpose(0, 3, 2, 1).reshape(8, 128, 4096))
            xyzT[b] = xyz[gb].T
            x2 = (xyz[gb].astype(np.float64) ** 2).sum(-1).astype(np.float32)
            for blk in range(2):
                i = 2 * b + blk
                # rows 5g+c over tokens (g, t) of this blk
                xs = xyz[gb][(blk * 8) * 512:(blk * 8 + 8) * 512]  # (8*512, 3)
                xs = xs.reshape(8, 512, 3)
                x2s = x2[(blk * 8) * 512:(blk * 8 + 8) * 512].reshape(8, 512)
                blkm = np.empty((8, 5, 512), np.float32)
                blkm[:, 0:3] = xs.transpose(0, 2, 1)
                blkm[:, 3] = x2s
                blkm[:, 4] = 1.0
                xp[:, i * 512:(i + 1) * 512] = blkm.reshape(40, 512)
        in_maps.append({
            "tokTe": np.ascontiguousarray(tokTe),
            "xyzT": np.ascontiguousarray(xyzT),
            "xp": np.ascontiguousarray(xp),
            "C5": C5, "W1p": W1, "W2p": W2p, "b1p": b1p, "b2bc": b2bc,
            "m2": m2, "mrep": mrep,
        })
    return in_maps


def _unlayout(arr):
    """[128, 1024] (p=16g+e, f=blk*512+t) -> (8192, 16)"""
    return (arr.reshape(8, 16, 2, 512).transpose(2, 0, 3, 1)
            .reshape(N, E))


def kernel(**inputs):
    from concourse.bass_utils import run_bass_kernel_spmd

    nc = _get_prog(_DEBUG)
    in_maps = make_in_maps(inputs)
    res = run_bass_kernel_spmd(nc, in_maps, list(range(N_CORES)))
    dispatch = np.empty((B, N, E), np.float32)
    combine = np.empty((B, N, E), np.float32)
    for c in range(N_CORES):
        for b in range(BPC):
            dispatch[BPC * c + b] = _unlayout(res.results[c]["disp"][b])
            combine[BPC * c + b] = _unlayout(res.results[c]["comb"][b])
    if _DEBUG:
        kernel._dbg = [(res.results[c]["dbg_logits"], res.results[c]["dbg_dists"])
                       for c in range(N_CORES)]
    return dispatch, combine
